# revision 1
# baseline (speedup 1.0000x reference)
"""Trainium2 Bass kernel for nn_BiMaTrLayer (dual-path filter + bidirectional
Mamba/attention stack + GLU).  Data-parallel over 8 NeuronCores (4 samples per
core, processed as 2 passes of 2 samples).

On-chip layout: activations are feature-major ("fm"): [128-partition d-chunks,
free = (sample, time)].  PE does all dense algebra (weights pre-transposed
host-side to contraction-major), ACT does transcendentals and PSUM evacuation,
DVE does elementwise plus the selective scan (tensor_tensor_scan), GPSIMD takes
the scan accumulator adds, DMA broadcasts B/C rows via a DRAM bounce.
"""

import sys
import numpy as np

sys.path.append("/opt/trn_rl_repo")

import concourse.bass as bass
from concourse import bacc


class _Bacc(bacc.Bacc):
    """Bacc with act-table steering: resolve Exp and Ln to the combined
    natural_log_exp_and_others set so softplus/LN chains don't ping-pong
    table loads (2.7us each)."""

    def insert_act_table_loads(self):
        import concourse.mybir as _mb
        from concourse.hw_specs import get_activation_tables
        from concourse import bacc as _bacc
        has_activation = any(
            isinstance(i, _mb.InstActivation)
            for b in self.main_func.blocks
            for i in b.instructions
        )
        if not has_activation:
            return
        tables = list(get_activation_tables(self.m.arch).items())
        AFT = _mb.ActivationFunctionType
        steer = {"exp_and_others": {AFT.Exp}, "exp_and_friends": {AFT.Exp},
                 "natural_log": {AFT.Ln}}
        tables = [(nm, fn - steer.get(nm, set())) for nm, fn in tables]
        _bacc._bass_rust.insert_act_table_loads(self, tables)

import concourse.mybir as mybir
import concourse.tile as tile
from concourse.masks import make_identity
from contextlib import ExitStack

AF = mybir.ActivationFunctionType
OP = mybir.AluOpType
F32 = mybir.dt.float32
BF16 = mybir.dt.bfloat16
SCAN_BF16 = True
P = 128

B, S, D = 32, 256, 256
NCORES = 8
BC = B // NCORES            # samples per core
PB = 2                      # samples per pass
NPASS = BC // PB
F = PB * S                  # 512: free dim (sample, time) per pass
DI, DS, DTR, NL, H, HD = 512, 16, 16, 2, 4, 64
DIC = DI // P
L2 = 69
NF = S // 2 + 1
DC = 4

DEC_LO = np.array([-0.010597401784997278, 0.032883011666982945,
                   0.030841381835986965, -0.18703481171888114,
                   -0.02798376941698385, 0.6308807679295904,
                   0.7148465705525415, 0.23037781330885523], np.float64)


# ----------------------------------------------------------------- host consts
def _dwt1_mat(L):
    out_full = L + 14 - 8 + 1
    idx = np.arange(1, out_full, 2)
    M = np.zeros((len(idx), L))
    for s in range(L):
        x = np.zeros(L)
        x[s] = 1.0
        y = np.correlate(np.pad(x, 7), DEC_LO[::-1], 'valid')
        M[:, s] = y[idx]
    return M


def _interp_mat(Lin, out_len):
    pos = (np.arange(out_len) + 0.5) * (Lin / out_len) - 0.5
    pos = np.clip(pos, 0.0, Lin - 1.0)
    lo = np.floor(pos).astype(int)
    hi = np.minimum(lo + 1, Lin - 1)
    t = pos - lo
    M = np.zeros((out_len, Lin))
    M[np.arange(out_len), lo] += 1.0 - t
    M[np.arange(out_len), hi] += t
    return M


def _fft_mats():
    s = np.arange(S)
    f = np.arange(NF)
    ang = 2 * np.pi * np.outer(f, s) / S
    Fr = np.cos(ang) / np.sqrt(S)
    Fi = -np.sin(ang) / np.sqrt(S)
    c = np.full(NF, 2.0)
    c[0] = 1.0
    c[-1] = 1.0
    angT = 2 * np.pi * np.outer(s, f) / S
    Gr = c * np.cos(angT) / np.sqrt(S)
    Gi = -c * np.sin(angT) / np.sqrt(S)
    Gi[:, 0] = 0.0
    Gi[:, -1] = 0.0
    return Fr, Fi, Gr, Gi


def _host_consts():
    Fr, Fi, Gr, Gi = _fft_mats()
    D1 = _dwt1_mat(S)
    D2 = _dwt1_mat(D1.shape[0])
    T = D2 @ D1
    I = _interp_mat(T.shape[0], S)
    f32 = lambda a: np.ascontiguousarray(a, np.float32)
    return dict(frT=f32(Fr.T), fiT=f32(Fi.T), grT=f32(Gr.T), giT=f32(Gi.T),
                tdT=f32(T.T), iiT=f32(I.T))


def _prep_weights(inp):
    f32 = lambda a: np.ascontiguousarray(np.asarray(a), np.float32)
    w = dict(_host_consts())
    w["fftWa"] = f32(np.concatenate([np.asarray(inp["fft_W"]).T,
                                     np.asarray(inp["fft_b"])[None, :]], 0))
    for nm in ("wl1", "wl2"):
        w[nm + "T"] = f32(np.asarray(inp[nm + "_W"]).transpose(2, 1, 0))
        w[nm + "b"] = f32(np.asarray(inp[nm + "_b"])[:, None])
    qkv = np.asarray(inp["ca_Wqkv"])
    bqkv = np.asarray(inp["ca_bqkv"])
    wo = np.asarray(inp["ca_Wo"])
    w["caWqT"] = f32(qkv[0:D].T)
    w["caWkT"] = f32(qkv[D:2 * D].T)
    w["caWvT"] = f32(qkv[2 * D:].T)
    w["caWoT"] = f32(wo.T)
    w["caBq"] = f32(bqkv[0:D][:, None])
    w["caBk"] = f32(bqkv[D:2 * D][:, None])
    w["caBo"] = f32((np.asarray(inp["ca_bo"]) + wo @ bqkv[2 * D:])[:, None])
    w["gateWT"] = f32(np.asarray(inp["gate_W"]).T)
    w["gateB"] = f32(np.asarray(inp["gate_b"])[:, None])
    for pre in ("mf", "mb"):
        w[pre + "inWT"] = f32(np.asarray(inp[pre + "_in_W"]).transpose(0, 2, 1))
        w[pre + "convW"] = f32(inp[pre + "_conv_W"])
        w[pre + "convB"] = f32(np.asarray(inp[pre + "_conv_b"])[:, :, None])
        w[pre + "cols"] = f32(np.stack([np.asarray(inp[pre + "_conv_b"]),
                                        np.asarray(inp[pre + "_dt_b"]),
                                        np.asarray(inp[pre + "_D"])], -1))
        xp = np.asarray(inp[pre + "_xproj_W"]).transpose(0, 2, 1)  # [NL, DI, 48]
        perm = list(range(DTR, DTR + 2 * DS)) + list(range(DTR))     # [B;C;dt]
        w[pre + "xpT"] = f32(xp[:, :, perm])
        w[pre + "dtWT"] = f32(np.asarray(inp[pre + "_dt_W"]).transpose(0, 2, 1))
        w[pre + "dtB"] = f32(np.asarray(inp[pre + "_dt_b"])[:, :, None])
        w[pre + "Alog"] = f32(inp[pre + "_Alog"])
        w[pre + "Dv"] = f32(np.asarray(inp[pre + "_D"])[:, :, None])
        w[pre + "outWT"] = f32(np.asarray(inp[pre + "_out_W"]).transpose(0, 2, 1))
    for pre in ("af", "ab"):
        qkv = np.asarray(inp[pre + "_Wqkv"])
        bqkv = np.asarray(inp[pre + "_bqkv"])
        wo = np.asarray(inp[pre + "_Wo"])
        w[pre + "WqT"] = f32(qkv[:, 0:D].transpose(0, 2, 1))
        w[pre + "WkT"] = f32(qkv[:, D:2 * D].transpose(0, 2, 1))
        w[pre + "WvT"] = f32(qkv[:, 2 * D:].transpose(0, 2, 1))
        w[pre + "WoT"] = f32(wo.transpose(0, 2, 1))
        w[pre + "Bq"] = f32(bqkv[:, 0:D][:, :, None])
        w[pre + "Bk"] = f32(bqkv[:, D:2 * D][:, :, None])
        w[pre + "Bo"] = f32((np.asarray(inp[pre + "_bo"])
                             + np.einsum('lod,ld->lo', wo, bqkv[:, 2 * D:]))[:, :, None])
    w["flG"] = f32(np.asarray(inp["fl_ln_g"])[None, :])
    w["flB"] = f32(np.asarray(inp["fl_ln_b"])[None, :])
    w["gluG"] = f32(np.asarray(inp["glu_ln_g"])[None, :])
    w["gluB"] = f32(np.asarray(inp["glu_ln_b"])[None, :])
    for nm in ("anf", "anb", "nf", "nb"):
        w[nm + "G"] = f32(np.asarray(inp[nm + "_g"])[:, None, :])
        w[nm + "B"] = f32(np.asarray(inp[nm + "_b"])[:, None, :])
    w["glu1WT"] = f32(np.asarray(inp["glu1_W"]).T)
    w["glu1B"] = f32(np.asarray(inp["glu1_b"])[:, None])
    w["glu2WT"] = f32(np.asarray(inp["glu2_W"]).T)
    w["glu2B"] = f32(np.asarray(inp["glu2_b"])[:, None])
    return w


# ----------------------------------------------------------------- emit helpers
class Emit:
    def __init__(self, nc, tc, ctx):
        self.nc, self.tc = nc, tc
        self.sb = ctx.enter_context(tc.tile_pool(name="sb", bufs=1))
        self.s2p = ctx.enter_context(tc.tile_pool(name="s2p", bufs=2))
        self.s3p = ctx.enter_context(tc.tile_pool(name="s3p", bufs=2))
        self.pp = ctx.enter_context(tc.tile_pool(name="pp", bufs=4, space="PSUM"))
        self.pt = ctx.enter_context(tc.tile_pool(name="pt", bufs=1, space="PSUM"))
        self.pn = ctx.enter_context(tc.tile_pool(name="pn", bufs=3, space="PSUM"))

    def load_wT(self, drh, K, M, tag, bufs2=False):
        nc = self.nc
        if not isinstance(drh, bass.AP):
            drh = drh[:, :]
        kc_n = (K + P - 1) // P
        pool = self.s2p if bufs2 else self.sb
        t = pool.tile([min(K, P), kc_n, M], F32, tag=tag)
        if K % P == 0:
            # one DMA: dram [K, M] -> sbuf [128, KC, M]
            st = drh.ap[-1][0]
            src = bass.AP(tensor=drh.tensor, offset=drh.offset,
                          ap=[[M * st, P], [P * M * st, kc_n], [st, M]])
            nc.sync.dma_start(out=t, in_=src)
        else:
            for kc in range(kc_n):
                kp = min(P, K - kc * P)
                nc.sync.dma_start(out=t[:kp, kc, :], in_=drh[kc * P:kc * P + kp, :])
        return t

    def load_col(self, drh, M, tag):
        nc = self.nc
        if not isinstance(drh, bass.AP):
            drh = drh[:, :]
        mc_n = (M + P - 1) // P
        t = self.sb.tile([P, mc_n], F32, tag=tag)
        if M % P == 0:
            src = bass.AP(tensor=drh.tensor, offset=drh.offset,
                          ap=[[1, P], [P, mc_n]])
            nc.sync.dma_start(out=t, in_=src)
        else:
            for mc in range(mc_n):
                mp = min(P, M - mc * P)
                nc.sync.dma_start(out=t[:mp, mc:mc + 1],
                                  in_=drh[mc * P:mc * P + mp, :])
        return t

    def dense(self, x, wT, Mout, bias=None, act=None, out=None, out_pool=None,
              out_tag=None, Fw=None):
        nc = self.nc
        Fw = Fw or F
        kc_n = x.shape[1]
        mc_n = (Mout + P - 1) // P
        if out is None:
            out = (out_pool or self.s3p).tile([P, mc_n, Fw], F32, tag=out_tag)
        for mc in range(mc_n):
            mp = min(P, Mout - mc * P)
            ps = self.pp.tile([P, 512], F32, tag="mm")
            for kc in range(kc_n):
                nc.tensor.matmul(ps[:mp, :Fw],
                                 wT[:, kc, mc * P:mc * P + mp],
                                 x[:, kc, 0:Fw],
                                 start=(kc == 0), stop=(kc == kc_n - 1))
            bap = bias[:mp, mc:mc + 1] if bias is not None else None
            if act is None and bias is None:
                nc.scalar.copy(out[:mp, mc, 0:Fw], ps[:mp, :Fw])
            else:
                nc.scalar.activation(out[:mp, mc, 0:Fw], ps[:mp, :Fw],
                                     act or AF.Identity,
                                     bias=bap if bap is not None else 0.0,
                                     scale=1.0)
        return out

    def add(self, out, a, b):
        self.nc.vector.tensor_add(out, a, b)

    def mul(self, out, a, b):
        self.nc.vector.tensor_mul(out, a, b)

    def act(self, out, in_, func, bias=0.0, scale=1.0):
        self.nc.scalar.activation(out=out, in_=in_, func=func, bias=bias, scale=scale)


def rev_view(ap2, n_blk, blk):
    st = ap2.ap[-1][0]
    off = ap2.offset + (blk - 1) * st
    if n_blk == 1:
        return bass.AP(tensor=ap2.tensor, offset=off, ap=[ap2.ap[0], [-st, blk]])
    return bass.AP(tensor=ap2.tensor, offset=off,
                   ap=[ap2.ap[0], [blk * st, n_blk], [-st, blk]])


def bcast_row(drh_row, parts):
    return bass.AP(tensor=drh_row.tensor, offset=drh_row.offset,
                   ap=[[0, parts]] + drh_row.ap[1:])


def _layer_norm(E, x, gR, bR, eps, out):
    """x, out: [128, 2, F] feature-major (D=256 on partitions)."""
    nc = E.nc
    gneg = E.sb.tile([1, D], F32, tag="lngn")
    nc.vector.tensor_scalar_mul(gneg, gR, -1.0)
    for f0 in range(0, F, 512):
        fw = min(512, F - f0)
        stat = E.sb.tile([1, 4, 512], F32, tag="thin8")
        m, q, var = (stat[0:1, i, :fw] for i in range(3))
        mr = q
        xsq = E.s2p.tile([P, 512], F32, tag="dA")
        for which, dst in ((0, m), (1, q)):
            ps = E.pn.tile([P, 512], F32, tag="th")
            for kc in range(2):
                src = x[:, kc, f0:f0 + fw]
                if which == 1:
                    E.act(xsq[:, :fw], src, AF.Square)
                    src = xsq[:, :fw]
                nc.tensor.matmul(ps[0:1, :fw], E.ones128, src,
                                 start=(kc == 0), stop=(kc == 1))
            nc.vector.tensor_scalar_mul(dst, ps[0:1, :fw], 1.0 / D)
        E.mul(var, m, m)
        nc.vector.tensor_tensor(var, q, var, OP.subtract)
        E.act(var, var, AF.Ln, bias=E.eps[eps][0:1, 0:1])
        E.act(var, var, AF.Exp, scale=-0.5)        # var row now holds r
        E.mul(mr, m, var)
        for mc in range(2):
            gRc = gR[0:1, mc * P:(mc + 1) * P]
            bRc = bR[0:1, mc * P:(mc + 1) * P]
            gnc = gneg[0:1, mc * P:(mc + 1) * P]
            ps_s = E.pn.tile([P, 512], F32, tag="th")
            nc.tensor.matmul(ps_s[:, :fw], gRc, var, start=True, stop=True)
            ps_o = E.pn.tile([P, 512], F32, tag="th")
            nc.tensor.matmul(ps_o[:, :fw], bRc, E.onesF[0:1, :fw],
                             start=True, stop=False)
            nc.tensor.matmul(ps_o[:, :fw], gnc, mr, start=False, stop=True)
            tmp = E.s2p.tile([P, 512], F32, tag="lntmp")
            E.mul(tmp[:, :fw], x[:, mc, f0:f0 + fw], ps_s[:, :fw])
            E.add(out[:, mc, f0:f0 + fw], tmp[:, :fw], ps_o[:, :fw])
    return out


def _attention(E, q_src, kv_src, wq, wk, wv, wo, bq, bk, bo, out_tag):
    """MHA over PB samples; q_src/kv_src [128, 2, F] fm.  Returns [128, 2, F]."""
    nc = E.nc
    ident = E.ident
    ofm = E.s3p.tile([P, 2, F], F32, tag="t8")     # unnormalized o, fm
    se = E.sb.tile([1, H, PB, S], F32, tag="thin8")
    for b in range(PB):
        qf = E.s2p.tile([P, 2, S], F32, tag="qfb")
        kf = E.s2p.tile([P, 2, S], F32, tag="kfb")
        vtm = E.s2p.tile([P, 2, D], F32, tag="vtmb")
        for mc in range(2):
            for dst, wT, bias in ((qf, wq, bq), (kf, wk, bk)):
                ps = E.pp.tile([P, 512], F32, tag="mm")
                for kc in range(2):
                    nc.tensor.matmul(ps[:, :S], wT[:, kc, mc * P:(mc + 1) * P],
                                     q_src[:, kc, b * S:(b + 1) * S] if dst is qf
                                     else kv_src[:, kc, b * S:(b + 1) * S],
                                     start=(kc == 0), stop=(kc == 1))
                nc.scalar.activation(dst[:, mc, :], ps[:, :S], AF.Identity,
                                     bias=bias[:, mc:mc + 1], scale=1.0)
        for tcn in range(2):
            ps = E.pp.tile([P, 512], F32, tag="mm")
            for kc in range(2):
                nc.tensor.matmul(ps[:, :D],
                                 kv_src[:, kc, b * S + tcn * P: b * S + (tcn + 1) * P],
                                 wv[:, kc, :], start=(kc == 0), stop=(kc == 1))
            nc.scalar.copy(vtm[:, tcn, :], ps[:, :D])
        pse = None
        for h in range(H):
            hc, off = h // 2, (h % 2) * 64
            expT = E.s2p.tile([P, 2, S], F32, tag="expT")
            for kc in range(2):
                ps = E.pp.tile([P, 512], F32, tag="mm")
                nc.tensor.matmul(ps[:, :S],
                                 kf[off:off + 64, hc, kc * P:(kc + 1) * P],
                                 qf[off:off + 64, hc, :],
                                 start=True, stop=True)
                E.act(expT[:, kc, :], ps[:, :S], AF.Exp, scale=1.0 / np.sqrt(HD))
            if h % 2 == 0:
                pse = E.pn.tile([P, 512], F32, tag="th")
            for kc in range(2):
                nc.tensor.matmul(pse[0:1, (h % 2) * S:(h % 2) * S + S],
                                 E.ones128, expT[:, kc, :],
                                 start=(kc == 0), stop=(kc == 1))
            if h % 2 == 1:
                E.act(se[0:1, h - 1:h + 1, b, :],
                      pse[0:1, :].rearrange("p (h s) -> p h s", h=2), AF.Ln)
            # o feature-major directly: out[dv, q] = sum_k vtm[k, dv] * expT[k, q]
            ps = E.pp.tile([P, 512], F32, tag="mm")
            for kc in range(2):
                nc.tensor.matmul(ps[:64, :S], vtm[:, kc, h * 64:(h + 1) * 64],
                                 expT[:, kc, :], start=(kc == 0), stop=(kc == 1))
            nc.scalar.copy(ofm[off:off + 64, hc, b * S:(b + 1) * S], ps[:64, :S])
    E.act(se, se, AF.Exp, scale=-1.0)              # 1/sumexp, in place
    for h in range(H):
        dc, off = h // 2, (h % 2) * 64
        ps = E.pn.tile([P, 512], F32, tag="th")
        nc.tensor.matmul(ps[0:64, :F], E.ones1x64,
                         se[0:1, h].rearrange("p b s -> p (b s)"),
                         start=True, stop=True)
        E.mul(ofm[off:off + 64, dc, :], ofm[off:off + 64, dc, :], ps[0:64, :F])
    return E.dense(ofm, wo, D, bias=bo, out_tag=out_tag)


def _mamba(E, io, x, pre, l, flip, bc_dram, out_tag):
    nc = E.nc
    inW = E.load_wT(io[pre + "inWT"][l], D, 2 * DI, "inW")
    cols = E.sb.tile([P, DIC, 3], F32, tag="mcols")
    cd = io[pre + "cols"][l]
    nc.sync.dma_start(out=cols, in_=bass.AP(
        tensor=cd.tensor, offset=cd.offset, ap=[[3, P], [P * 3, DIC], [1, 3]]))
    cw = E.sb.tile([P, DIC, DC], F32, tag="cw")
    cwd = io[pre + "convW"][l]
    nc.sync.dma_start(out=cw, in_=bass.AP(
        tensor=cwd.tensor, offset=cwd.offset,
        ap=[[DC, P], [P * DC, DIC], [1, DC]]))
    xc = E.sb.tile([P, DIC, F], F32, tag="xcz")
    for c in range(DIC):
        xi = E.s2p.tile([P, F], F32, tag="xib")
        ps = E.pp.tile([P, 512], F32, tag="mm")
        for b in range(PB):
            for kc in range(2):
                rhs = x[:, kc, b * S:(b + 1) * S]
                if flip:
                    rhs = rev_view(rhs, 1, S)
                nc.tensor.matmul(ps[:, b * S:(b + 1) * S],
                                 inW[:, kc, c * P:(c + 1) * P], rhs,
                                 start=(kc == 0), stop=(kc == 1))
        nc.scalar.copy(xi, ps)
        diag = E.sb.tile([P, DC, P], F32, tag="diag")
        for j in range(DC):
            nc.vector.tensor_scalar_mul(diag[:, j, :], E.ident, cw[:, c, j:j + 1])
        ps = E.pp.tile([P, 512], F32, tag="mm")
        for b in range(PB):
            nc.tensor.matmul(ps[:, b * S:(b + 1) * S], diag[:, DC - 1, :],
                             xi[:, b * S:(b + 1) * S], start=True, stop=False)
            for j in range(DC - 1):
                sh = DC - 1 - j
                nc.tensor.matmul(ps[:, b * S + sh:(b + 1) * S], diag[:, j, :],
                                 xi[:, b * S:(b + 1) * S - sh],
                                 start=False, stop=(j == DC - 2))
        nc.scalar.activation(xc[:, c, :], ps, AF.Identity,
                             bias=cols[:, c, 0:1], scale=1.0)
    E.act(xc, xc, AF.Silu)
    xpw = E.load_wT(io[pre + "xpT"][l], DI, DTR + 2 * DS, "xpw")
    dbl = E.sb.tile([DTR + 2 * DS, F], F32, tag="dbl")
    ps = E.pp.tile([P, 512], F32, tag="mm")
    for kc in range(DIC):
        nc.tensor.matmul(ps[:DTR + 2 * DS, :F], xpw[:, kc, :], xc[:, kc, :],
                         start=(kc == 0), stop=(kc == DIC - 1))
    nc.scalar.copy(dbl, ps[:DTR + 2 * DS, :F])
    dtw = E.sb.tile([2 * DS + DTR, DI], F32, tag="dtw")
    nc.sync.dma_start(out=dtw[2 * DS:, :], in_=io[pre + "dtWT"][l])
    dt = E.sb.tile([P, DIC, F], F32, tag="dt")
    for mc in range(DIC):
        ps = E.pp.tile([P, 512], F32, tag="mm")
        nc.tensor.matmul(ps[:, :F], dtw[2 * DS:, mc * P:(mc + 1) * P], dbl[2 * DS:2 * DS + DTR, :],
                         start=True, stop=True)
        # softplus(x + b) = ln(1 + exp(x + b)); softplus has no HW act table
        dtx = E.s2p.tile([P, F], F32, tag="dA")
        E.act(dtx, ps[:, :F], AF.Exp, bias=cols[:, mc, 1:2])
        E.act(dt[:, mc, :], dtx, AF.Ln, bias=1.0)
    Aneg = E.sb.tile([P, DIC, DS], F32, tag="Aneg")
    ald = io[pre + "Alog"][l]
    nc.sync.dma_start(out=Aneg, in_=bass.AP(
        tensor=ald.tensor, offset=ald.offset,
        ap=[[DS, P], [P * DS, DIC], [1, DS]]))
    E.act(Aneg, Aneg, AF.Exp)
    nc.vector.tensor_scalar_mul(Aneg, Aneg, -1.0)
    SDT = BF16 if SCAN_BF16 else F32
    dtu = E.sb.tile([P, DIC, F], SDT, tag="dtu")
    E.mul(dtu, dt, xc)
    y = E.sb.tile([P, DIC, F], F32, tag="yac")
    for c in range(DIC):
        nc.vector.tensor_scalar_mul(y[:, c, :], xc[:, c, :], cols[:, c, 2:3])
    # bounce B/C rows (possibly bf16) through DRAM for partition broadcast
    if SCAN_BF16:
        dblbc = E.sb.tile([2 * DS, F], BF16, tag="dblbc")
        nc.vector.tensor_copy(dblbc, dbl[0:2 * DS, :])
        nc.sync.dma_start(out=bc_dram[:, :], in_=dblbc)
    else:
        nc.sync.dma_start(out=bc_dram[:, :], in_=dbl[0:2 * DS, :])
    # poison sample-start columns of dt so exp(dt*A) -> 0 there (state reset);
    # dtu/y-init already read the true dt values above
    nc.vector.memset(dt[:, :, 0:F:S], 1.0e30)
    rep = lambda t2: bass.AP(tensor=t2.tensor, offset=t2.offset,
                             ap=[t2.ap[0], [0, DIC]] + t2.ap[1:])
    ysc = E.sb.tile([P, DIC, F], SDT, tag="ysc")
    for n in range(DS):
        Bb = E.s2p.tile([P, F], SDT, tag="Bb")
        Cb = E.s2p.tile([P, F], SDT, tag="Cb")
        nc.gpsimd.dma_start(out=Bb, in_=bcast_row(bc_dram[n:n + 1, :], P))
        nc.scalar.dma_start(out=Cb, in_=bcast_row(bc_dram[DS + n:DS + n + 1, :], P))
        dBu = E.s2p.tile([P, DIC, F], SDT, tag="dBu")
        E.mul(dBu, dtu, rep(Bb[:, :]))
        hn = E.s2p.tile([P, DIC, F], SDT, tag="hn")
        for c in range(DIC):
            dA = E.s2p.tile([P, F], F32, tag="dA")
            E.act(dA, dt[:, c, :], AF.Exp, scale=Aneg[:, c, n:n + 1])
            nc.vector.tensor_tensor_scan(out=hn[:, c, :], data0=dA,
                                         data1=dBu[:, c, :],
                                         initial=0.0, op0=OP.mult, op1=OP.add)
        E.mul(hn, hn, rep(Cb[:, :]))
        if n == 0:
            nc.vector.tensor_copy(ysc, hn)
        else:
            E.add(ysc, ysc, hn)
    E.add(y, y, ysc)
    # y * silu(z): z chunks re-derived from in-proj (chunks DIC..2*DIC-1);
    # z reuses the xc slot (dead once dtu/D-init are done) so its matmuls and
    # silu overlap the scan instead of trailing it
    z = E.sb.tile([P, DIC, F], F32, tag="xcz")
    for c in range(DIC):
        ps = E.pp.tile([P, 512], F32, tag="mm")
        for b in range(PB):
            for kc in range(2):
                rhs = x[:, kc, b * S:(b + 1) * S]
                if flip:
                    rhs = rev_view(rhs, 1, S)
                nc.tensor.matmul(ps[:, b * S:(b + 1) * S],
                                 inW[:, kc, (DIC + c) * P:(DIC + c + 1) * P],
                                 rhs, start=(kc == 0), stop=(kc == 1))
        nc.scalar.copy(z[:, c, :], ps)
    E.act(z, z, AF.Silu)
    E.mul(y, y, z)
    ow = E.load_wT(io[pre + "outWT"][l], DI, D, "outW")
    return E.dense(y, ow, D, out_pool=E.s2p, out_tag=out_tag)


# ------------------------------------------------------------------- program
def build_program(wshapes):
    nc = _Bacc()
    io = {}
    io["input"] = nc.declare_dram_parameter("input", [BC, S, D], F32, isOutput=False)
    for k, shp in wshapes.items():
        io[k] = nc.declare_dram_parameter(k, list(shp), F32, isOutput=False)
    io["out"] = nc.declare_dram_parameter("out", [BC, S, D], F32, isOutput=True)
    bc_dram = [nc.dram_tensor(f"bcrows{i}", [2 * DS, F], BF16 if SCAN_BF16 else F32)
               for i in range(NPASS * NL * 2)]
    with tile.TileContext(nc) as tc:
        with ExitStack() as ctx:
            E = Emit(nc, tc, ctx)
            ident = E.sb.tile([P, P], F32, tag="ident")
            make_identity(nc, ident)
            E.ident = ident
            E.ones128 = E.sb.tile([P, 1], F32, tag="ones128")
            nc.vector.memset(E.ones128, 1.0)
            E.ones1x64 = E.sb.tile([1, 64], F32, tag="ones64")
            nc.vector.memset(E.ones1x64, 1.0)
            E.ones1xP = E.sb.tile([1, P], F32, tag="ones1p")
            nc.vector.memset(E.ones1xP, 1.0)
            E.onesF = E.sb.tile([1, 512], F32, tag="onesF")
            nc.vector.memset(E.onesF, 1.0)
            E.eps = {}
            for ev in (1e-5, 1e-12):
                t = E.sb.tile([1, 1], F32, tag=f"eps{ev}")
                nc.vector.memset(t, ev)
                E.eps[ev] = t
            for p in range(NPASS):
                _emit_pass(E, io, p, bc_dram[p * NL * 2:(p + 1) * NL * 2])
    nc.finalize()
    return nc


def _emit_pass(E, io, pss, bc_dram):
    nc = E.nc
    ident = E.ident

    # ---------------- stage 0: load x + transpose to feature-major
    x_tm = E.sb.tile([P, PB * 2, D], F32, tag="xtm")
    for b in range(PB):
        for sc in range(2):
            nc.sync.dma_start(out=x_tm[:, b * 2 + sc, :],
                              in_=io["input"][pss * PB + b, sc * P:(sc + 1) * P, :])
    x_fm = E.sb.tile([P, 2, F], F32, tag="xfm")
    for b in range(PB):
        for sc in range(2):
            for dc in range(2):
                pst = E.pt.tile([P, P], F32, tag="tp")
                nc.tensor.transpose(pst, x_tm[:, b * 2 + sc, dc * P:(dc + 1) * P], ident)
                nc.scalar.copy(x_fm[:, dc, b * S + sc * P: b * S + (sc + 1) * P], pst)

    # ---------------- stage 1: FFT path
    frT = E.load_wT(io["frT"], S, NF, "frT")
    fiT = E.load_wT(io["fiT"], S, NF, "fiT")
    fftWa = E.load_wT(io["fftWa"], 513, 2 * D, "inW")
    grT = E.load_wT(io["grT"], NF, S, "grT")
    giT = E.load_wT(io["giT"], NF, S, "giT")
    x_fft = E.sb.tile([P, 2, F], F32, tag="qfb2")
    for b in range(PB):
        comb = E.s3p.tile([P, 4, NF], F32, tag="t8")
        for ri, mat in ((0, frT), (1, fiT)):
            for mc in range(2):
                ps = E.pp.tile([P, 512], F32, tag="mm")
                for kc in range(2):
                    nc.tensor.matmul(ps[:, :NF], x_tm[:, b * 2 + kc, mc * P:(mc + 1) * P],
                                     mat[:, kc, :], start=(kc == 0), stop=(kc == 1))
                nc.scalar.copy(comb[:, ri * 2 + mc, :], ps[:, :NF])
        filt = E.s3p.tile([P, 2 * D], F32, tag="t8")
        filtN = E.sb.tile([1, 2 * D], F32, tag="filtN")
        for mt, mp, f0 in ((filt, P, 0), (filtN, 1, P)):
            ps = E.pp.tile([P, 512], F32, tag="mm")
            for kc in range(4):
                nc.tensor.matmul(ps[:mp, :], comb[:, kc, f0:f0 + mp], fftWa[:, kc, :],
                                 start=(kc == 0), stop=False)
            nc.tensor.matmul(ps[:mp, :], E.ones1xP[0:1, 0:mp], fftWa[0:1, 4, :],
                             start=False, stop=True)
            E.act(mt[0:mp, :] if mt is filtN else mt, ps[:mp, :], AF.Gelu)
        for mc in range(2):
            ps = E.pp.tile([P, 512], F32, tag="mm")
            nc.tensor.matmul(ps[:, :S], filt[:, mc * P:(mc + 1) * P], grT[:, 0, :],
                             start=True, stop=False)
            nc.tensor.matmul(ps[:, :S], filtN[0:1, mc * P:(mc + 1) * P], grT[0:1, 1, :],
                             start=False, stop=False)
            nc.tensor.matmul(ps[:, :S], filt[:, D + mc * P:D + (mc + 1) * P], giT[:, 0, :],
                             start=False, stop=False)
            nc.tensor.matmul(ps[:, :S], filtN[0:1, D + mc * P:D + (mc + 1) * P],
                             giT[0:1, 1, :], start=False, stop=True)
            nc.scalar.copy(x_fft[:, mc, b * S:(b + 1) * S], ps[:, :S])

    # ---------------- stage 2: wavelet path
    tdT = E.load_wT(io["tdT"], S, L2, "tdT")
    iiT = E.sb.tile([L2, S], F32, tag="iiT")
    nc.sync.dma_start(out=iiT, in_=io["iiT"][:, :])
    wl1T = [E.load_wT(io["wl1T"][k], D, D, t) for k, t in enumerate(("awq", "awk", "awv"))]
    wl2T = [E.load_wT(io["wl2T"][k], D, D, t) for k, t in enumerate(("awo", "outW", "xpw"))]
    wl1b = E.load_col(io["wl1b"], D, "wl1b")
    wl2b = E.load_col(io["wl2b"], D, "wl2b")
    x_wl = E.sb.tile([P, 2, F], F32, tag="kfb2")
    a_fm = E.sb.tile([P, 2, PB, L2], F32, tag="afm")
    for b in range(PB):
        for mc in range(2):
            ps = E.pp.tile([P, 512], F32, tag="mm")
            for kc in range(2):
                nc.tensor.matmul(ps[:, :L2], x_tm[:, b * 2 + kc, mc * P:(mc + 1) * P],
                                 tdT[:, kc, :], start=(kc == 0), stop=(kc == 1))
            nc.scalar.copy(a_fm[:, mc, b, :], ps[:, :L2])

    def conv3(src, wT, bcol, actf, dst_tag):
        dst = E.s2p.tile([P, 2, PB, L2], F32, tag=dst_tag)
        for b in range(PB):
            for mc in range(2):
                ps = E.pp.tile([P, 512], F32, tag="mm")
                for kc in range(2):
                    nc.tensor.matmul(ps[:, :L2], wT[1][:, kc, mc * P:(mc + 1) * P],
                                     src[:, kc, b, :], start=(kc == 0), stop=False)
                for kc in range(2):
                    nc.tensor.matmul(ps[:, 1:L2], wT[0][:, kc, mc * P:(mc + 1) * P],
                                     src[:, kc, b, 0:L2 - 1], start=False, stop=False)
                for kc in range(2):
                    nc.tensor.matmul(ps[:, 0:L2 - 1], wT[2][:, kc, mc * P:(mc + 1) * P],
                                     src[:, kc, b, 1:L2], start=False, stop=(kc == 1))
                E.act(dst[:, mc, b, :], ps[:, :L2], actf, bias=bcol[:, mc:mc + 1])
        return dst

    c1 = conv3(a_fm, wl1T, wl1b, AF.Gelu, "c1")
    c2 = conv3(c1, wl2T, wl2b, AF.Identity, "afm")
    c2T = E.sb.tile([L2, 2, PB, P], F32, tag="c2T")
    for b in range(PB):
        for mc in range(2):
            pst = E.pt.tile([P, P], F32, tag="tp")
            nc.tensor.transpose(pst[0:L2, :], c2[:, mc, b, :], ident)
            nc.scalar.copy(c2T[:, mc, b, :], pst[0:L2, :])
    for b in range(PB):
        for mc in range(2):
            ps = E.pp.tile([P, 512], F32, tag="mm")
            nc.tensor.matmul(ps[:, :S], c2T[:, mc, b, :], iiT, start=True, stop=True)
            nc.scalar.copy(x_wl[:, mc, b * S:(b + 1) * S], ps[:, :S])

    # ---------------- stage 3: cross-attention + gate + LN
    caWq = E.load_wT(io["caWqT"], D, D, "awq")
    caWk = E.load_wT(io["caWkT"], D, D, "awk")
    caWv = E.load_wT(io["caWvT"], D, D, "awv")
    caWo = E.load_wT(io["caWoT"], D, D, "awo")
    caBq = E.load_col(io["caBq"], D, "abq")
    caBk = E.load_col(io["caBk"], D, "abk")
    caBo = E.load_col(io["caBo"], D, "abo")
    att = _attention(E, x_fft, x_wl, caWq, caWk, caWv, caWo, caBq, caBk, caBo, "t8")
    fused = E.s3p.tile([P, 2, F], F32, tag="t8")
    E.add(fused, att, x_fm)
    gateW = E.load_wT(io["gateWT"], 2 * D, 2 * D, "bigw")
    gateB = E.load_col(io["gateB"], 2 * D, "bigb")
    ga = E.s3p.tile([P, 2, F], F32, tag="t8")
    gb = E.s3p.tile([P, 2, F], F32, tag="t8")
    for mc in range(4):
        actf = AF.Identity if mc < 2 else AF.Sigmoid
        gdst = ga if mc < 2 else gb
        ps = E.pp.tile([P, 512], F32, tag="mm")
        for kc in range(4):
            gsrc = fused if kc < 2 else x_fm
            nc.tensor.matmul(ps[:, :F], gateW[:, kc, mc * P:(mc + 1) * P],
                             gsrc[:, kc % 2, :], start=(kc == 0), stop=(kc == 3))
        E.act(gdst[:, mc % 2, :], ps[:, :F], actf, bias=gateB[:, mc:mc + 1])
    gated = ga
    E.mul(gated, ga, gb)
    flG = E.s2p.tile([1, D], F32, tag="lnG")
    flB = E.s2p.tile([1, D], F32, tag="lnB")
    nc.sync.dma_start(out=flG, in_=io["flG"][:, :])
    nc.sync.dma_start(out=flB, in_=io["flB"][:, :])
    x1 = E.s2p.tile([P, 2, F], F32, tag="x1")
    _layer_norm(E, gated, flG, flB, 1e-5, x1)

    # ---------------- stage 4: layers
    def ln_params(name, l):
        gt = E.s2p.tile([1, D], F32, tag="lnG")
        bt = E.s2p.tile([1, D], F32, tag="lnB")
        nc.sync.dma_start(out=gt, in_=io[name + "G"][l])
        nc.sync.dma_start(out=bt, in_=io[name + "B"][l])
        return gt, bt

    for l in range(NL):
        fs_keep = None
        for di, (mp, ap_, flip, anG, nG) in enumerate((
                ("mf", "af", False, "anf", "nf"),
                ("mb", "ab", True, "anb", "nb"))):
            ms = _mamba(E, io, x1, mp, l, flip, bc_dram[l * 2 + di], "ms")
            wq = E.load_wT(io[ap_ + "WqT"][l], D, D, "awq")
            wk = E.load_wT(io[ap_ + "WkT"][l], D, D, "awk")
            wv = E.load_wT(io[ap_ + "WvT"][l], D, D, "awv")
            wo = E.load_wT(io[ap_ + "WoT"][l], D, D, "awo")
            abq = E.load_col(io[ap_ + "Bq"][l], D, "abq")
            abk = E.load_col(io[ap_ + "Bk"][l], D, "abk")
            abo = E.load_col(io[ap_ + "Bo"][l], D, "abo")
            att = _attention(E, ms, ms, wq, wk, wv, wo, abq, abk, abo, "t8")
            s2 = E.s3p.tile([P, 2, F], F32, tag="t8")
            E.add(s2, ms, att)
            s3 = E.s3p.tile([P, 2, F], F32, tag="t8")
            ang, anb_ = ln_params(anG, l)
            _layer_norm(E, s2, ang, anb_, 1e-5, s3)
            s4 = E.s3p.tile([P, 2, F], F32, tag="t8")
            if flip:
                for kc in range(2):
                    E.add(s4[:, kc, :].rearrange("p (b s) -> p b s", b=PB),
                          rev_view(s3[:, kc, :], PB, S),
                          x1[:, kc, :].rearrange("p (b s) -> p b s", b=PB))
            else:
                E.add(s4, s3, x1)
            s5 = E.s2p.tile([P, 2, F], F32, tag="s5")
            ng, nb_ = ln_params(nG, l)
            _layer_norm(E, s4, ng, nb_, 1e-5, s5)
            if not flip:
                fs_keep = s5
        x1n = E.s2p.tile([P, 2, F], F32, tag="x1")
        E.add(x1n, fs_keep, s5)
        x1 = x1n

    # ---------------- stage 5: GLU + final LN
    glu1W = E.load_wT(io["glu1WT"], D, 2 * D, "bigw")
    glu1B = E.load_col(io["glu1B"], 2 * D, "bigb")
    va = E.s3p.tile([P, 2, F], F32, tag="t8")
    vb = E.s3p.tile([P, 2, F], F32, tag="t8")
    for mc in range(4):
        actf = AF.Identity if mc < 2 else AF.Sigmoid
        vdst = va if mc < 2 else vb
        ps = E.pp.tile([P, 512], F32, tag="mm")
        for kc in range(2):
            nc.tensor.matmul(ps[:, :F], glu1W[:, kc, mc * P:(mc + 1) * P],
                             x1[:, kc, :], start=(kc == 0), stop=(kc == 1))
        E.act(vdst[:, mc % 2, :], ps[:, :F], actf, bias=glu1B[:, mc:mc + 1])
    gv = va
    E.mul(gv, va, vb)
    glu2W = E.load_wT(io["glu2WT"], D, D, "bigw")
    glu2B = E.load_col(io["glu2B"], D, "bigb")
    gvo = E.dense(gv, glu2W, D, bias=glu2B, out_tag="t8")
    res = E.s3p.tile([P, 2, F], F32, tag="t8")
    E.add(res, gvo, x1)
    gluG = E.s2p.tile([1, D], F32, tag="lnG")
    gluB = E.s2p.tile([1, D], F32, tag="lnB")
    nc.sync.dma_start(out=gluG, in_=io["gluG"][:, :])
    nc.sync.dma_start(out=gluB, in_=io["gluB"][:, :])
    out_fm = E.s3p.tile([P, 2, F], F32, tag="t8")
    _layer_norm(E, res, gluG, gluB, 1e-12, out_fm)

    # ---------------- stage 6: transpose + store
    for b in range(PB):
        for sc in range(2):
            ot = E.sb.tile([P, D], F32, tag="otile")
            for dc in range(2):
                pst = E.pt.tile([P, P], F32, tag="tp")
                nc.tensor.transpose(pst, out_fm[:, dc, b * S + sc * P: b * S + (sc + 1) * P],
                                    ident)
                nc.scalar.copy(ot[:, dc * P:(dc + 1) * P], pst)
            nc.sync.dma_start(out=io["out"][pss * PB + b, sc * P:(sc + 1) * P, :], in_=ot)


# ------------------------------------------------------------------- driver
_CACHE = {}


def _get_program(wshapes):
    key = tuple(sorted((k, tuple(v)) for k, v in wshapes.items()))
    if key not in _CACHE:
        _CACHE[key] = build_program(wshapes)
    return _CACHE[key]


def kernel(**inputs):
    from concourse.bass_utils import run_bass_kernel_spmd
    w = _prep_weights(inputs)
    nc = _get_program({k: list(v.shape) for k, v in w.items()})
    x = np.ascontiguousarray(np.asarray(inputs["input_tensor"], np.float32))
    in_maps = []
    for core in range(NCORES):
        m = {"input": np.ascontiguousarray(x[core * BC:(core + 1) * BC])}
        m.update(w)
        in_maps.append(m)
    res = run_bass_kernel_spmd(nc, in_maps, list(range(NCORES)))
    return np.concatenate([res.results[i]["out"] for i in range(NCORES)], axis=0)

